# revision 12
# baseline (speedup 1.0000x reference)
"""Causal self-attention on 8 trn2 cores — v2.

Sharding: core c = 2*b + g handles batch b (of 4) and head group g (of 2,
8 heads each).  Tensor-parallel over heads for qkv/proj; host sums the two
w_proj partials per batch.

v2 structure (vs v1): heads processed in PAIRS (head 2p at partitions 0-63,
head 2p+1 at 64-127 of the qkT tiles), with

  - QK matmuls row-packed: the two K=64 stationaries sit at base partition
    0 / 64 so they land in disjoint PE array row groups and the two heads'
    S^T tiles compute concurrently.
  - attention loop per (pair, q-block jq of 512, key tile kt of 128):
    QK -> exp (narrowed to causal-valid cols, both heads in one ACT
    instruction) -> diag-block mask multiply on gpsimd -> PV accumulate
    into acc[65, 512] (65th row = softmax denominator via ones column in v).
  - softmax normalization DEFERRED: denominator rows gathered into a
    [32, 512] tile (partition-spread), one vector reciprocal per pair,
    broadcast at normalization time by a selector matmul, applied by one
    DVE multiply per (pair, jq) just before the projection consumes it.
  - the NEXT pair's qkT projection matmuls are interleaved into the
    attention loop (PE slack under the ACT-bound exp stream); x^T is
    streamed through a 16-slot ring and re-streamed per window (SBUF).
  - yraw tiles reuse dead kT tiles (SBUF budget).

All matmul operands float32r.
"""

import numpy as np

import concourse.bacc as bacc
import concourse.bass as bass
import concourse.tile as tile
import concourse.mybir as mybir
from concourse.bass_utils import run_bass_kernel_spmd

F32 = mybir.dt.float32
F32R = mybir.dt.float32r
AF = mybir.ActivationFunctionType

B, T, C, H = 4, 2048, 1024, 16
D = C // H  # 64
HG = 8  # heads per core
NCB = C // 128  # 8 contraction chunks
NTB = T // 512  # 4 t blocks
NKT = T // 128  # 16 key tiles
SCALE = 1.0 / 8.0  # 1/sqrt(D)

_cache = {}


def build_nc():
    if "nc" in _cache:
        return _cache["nc"]
    nc = bacc.Bacc("TRN2", target_bir_lowering=False, debug=False, num_devices=8)

    xt_d = nc.dram_tensor("xt", [C, T], F32, kind="ExternalInput").ap()
    wqk_d = nc.dram_tensor("wqk", [C, 2 * HG * D], F32, kind="ExternalInput").ap()
    wv_d = nc.dram_tensor("wv", [C, HG * D], F32, kind="ExternalInput").ap()
    wp_d = nc.dram_tensor("wp", [HG * D, C], F32, kind="ExternalInput").ap()
    masks_d = nc.dram_tensor("masks", [128, 4 * 512], F32, kind="ExternalInput").ap()
    sel_d = nc.dram_tensor("sel", [32, 32 * 64], F32, kind="ExternalInput").ap()
    ones_d = nc.dram_tensor("ones", [128, HG], F32, kind="ExternalInput").ap()
    out_d = nc.dram_tensor("out", [T, C], F32, kind="ExternalOutput").ap()

    with tile.TileContext(nc) as tc:
        with tc.tile_pool(name="persist", bufs=1) as persist:
            masks4 = persist.tile([128, 4, 512], F32R, name="masks4")
            nc.sync.dma_start(
                masks4, masks_d.rearrange("p (m w) -> p m w", m=4).bitcast(F32R)
            )
            sel3 = persist.tile([32, 16, 128], F32R, name="sel3")
            nc.sync.dma_start(
                sel3, sel_d.rearrange("p (r e) -> p r e", r=16).bitcast(F32R)
            )
            dens32 = persist.tile([32, 512], F32, name="dens32")
            recs32 = persist.tile([32, 512], F32R, name="recs32")

            # 9 big [128, T] tiles: qT p=0..3, kT p=0..3, one spare.  yraw of
            # pair p reuses the kT tile of pair p-1 (dead by then); yraw0
            # uses the spare.
            big = [
                persist.tile([128, T], F32R, tag=f"big{j}", name=f"big{j}")
                for j in range(9)
            ]
            qT = big[0:4]
            kT = big[4:8]
            yraw = [big[8], big[4], big[5], big[6]]

            v_sb = [
                persist.tile([128, HG, 65], F32R, tag=f"v{t}", name=f"v{t}")
                for t in range(NKT)
            ]
            ones_src = ones_d.rearrange("p (h o) -> p h o", o=1).bitcast(F32R)
            for tt in range(NKT):
                nc.sync.dma_start(v_sb[tt][:, :, 64:65], ones_src)

            with (
                tc.tile_pool(name="xrp", bufs=16) as xrp,
                tc.tile_pool(name="wqkp", bufs=4) as wqkp,
            ):
                # x^T streamed through a ring of 16 [128, 512] chunk tiles
                # (2 t-blocks deep); re-streamed once per qknext window.
                xring = {}

                def xr_dma(key, tb):
                    tiles = []
                    for cb in range(NCB):
                        t = xrp.tile([128, 512], F32R, tag="xr", name="xr")
                        nc.sync.dma_start(
                            t,
                            xt_d[
                                128 * cb : 128 * (cb + 1), 512 * tb : 512 * (tb + 1)
                            ].bitcast(F32R),
                        )
                        tiles.append(t)
                    xring[key] = tiles

                wqk_sl = {}

                def dma_wqk(jt):
                    t = wqkp.tile([128, NCB, 128], F32R, tag="wqk", name=f"wqk{jt}")
                    nc.sync.dma_start(
                        t,
                        wqk_d[:, 128 * jt : 128 * (jt + 1)]
                        .rearrange("(cb p) j -> p cb j", p=128)
                        .bitcast(F32R),
                    )
                    wqk_sl[jt] = t

                for jt in (0, 4, 1, 5):
                    dma_wqk(jt)

                # ---- upfront: v for all heads + qk for pair 0, per tb ----
                with (
                    tc.tile_pool(name="wvp", bufs=1) as wvp,
                    tc.tile_pool(name="ps1", bufs=4, space="PSUM") as ps1,
                ):
                    wv_sb = wvp.tile([128, NCB, 512], F32R, name="wv_sb")
                    nc.sync.dma_start(
                        wv_sb, wv_d.rearrange("(cb p) j -> p cb j", p=128).bitcast(F32R)
                    )
                    xr_dma(("u", 0), 0)
                    xr_dma(("u", 1), 1)
                    for tb in range(NTB):
                        if tb + 2 < NTB:
                            xr_dma(("u", tb + 2), tb + 2)
                        xc = xring[("u", tb)]
                        for t4 in range(4):
                            tt = 4 * tb + t4
                            ps = ps1.tile([128, 512], F32, tag="ps1", name="ps")
                            for cb in range(NCB):
                                nc.tensor.matmul(
                                    ps,
                                    xc[cb][:, 128 * t4 : 128 * (t4 + 1)],
                                    wv_sb[:, cb, :],
                                    start=(cb == 0),
                                    stop=(cb == NCB - 1),
                                )
                            nc.scalar.copy(
                                v_sb[tt][:, :, 0:64],
                                ps[:].rearrange("p (h e) -> p h e", h=HG),
                            )
                        for jt in (0, 4):
                            ps = ps1.tile([128, 512], F32, tag="ps1", name="ps")
                            for cb in range(NCB):
                                nc.tensor.matmul(
                                    ps,
                                    wqk_sl[jt][:, cb, :],
                                    xc[cb],
                                    start=(cb == 0),
                                    stop=(cb == NCB - 1),
                                )
                            nc.vector.tensor_copy(
                                big[jt][:, 512 * tb : 512 * (tb + 1)], ps
                            )

                # ---- attention: pairs of heads ----
                with (
                    tc.tile_pool(name="pp", bufs=2) as pp,
                    tc.tile_pool(name="stg", bufs=2) as stgp,
                    tc.tile_pool(name="strip", bufs=1, space="PSUM") as stripp,
                    tc.tile_pool(name="accp", bufs=1, space="PSUM") as accp,
                    tc.tile_pool(name="qnps", bufs=2, space="PSUM") as qnps,
                ):
                    strips = stripp.tile([128, 2, 1024], F32, name="strips")
                    accm = accp.tile([128, 2, 512], F32, name="accm")
                    # zero the two P slots once: narrowed exp leaves a stale
                    # prefix on diag tiles; after round one the stale data is
                    # old exp output (finite), but fresh SBUF could be NaN.
                    for _ in range(2):
                        Pz = pp.tile([128, 2, 512], F32R, tag="P", name="Pz")
                        nc.gpsimd.memset(Pz.bitcast(mybir.dt.uint32), 0)

                    for p in range(4):
                        hA, hB = 2 * p, 2 * p + 1
                        qt, kt_t = qT[p], kT[p]
                        # stream in wqk slices for the window after next
                        if p < 2:
                            dma_wqk(p + 2)
                            dma_wqk(6 + p)
                        # next-pair qk work interleaved into this pair's slack
                        qn_chunks = []
                        if p < 3:
                            for tb in range(NTB):
                                for jt in (p + 1, 5 + p):
                                    qn_chunks.append((jt, tb))
                            xr_dma((p, 0), 0)
                            xr_dma((p, 1), 1)
                        nqn = len(qn_chunks)  # 8 or 0
                        ntask = 40
                        qi = 0
                        ti = 0
                        si = 0
                        for jq in range(4):
                            nkt_q = 4 * jq + 4
                            for kt in range(nkt_q):
                                diag = (kt // 4) == jq
                                m = kt % 4 if diag else 0
                                off = 128 * m if diag else 0
                                s = si % 2
                                si += 1
                                P = pp.tile([128, 2, 512], F32R, tag="P", name="P")
                                # QK pair, row-packed (base partitions 0 / 64)
                                qs = 512 * jq
                                nc.tensor.matmul(
                                    strips[0:128, s, off:512],
                                    kt_t[0:64, 128 * kt : 128 * (kt + 1)],
                                    qt[0:64, qs + off : qs + 512],
                                    start=True,
                                    stop=True,
                                )
                                nc.tensor.matmul(
                                    strips[0:128, s, 512 + off : 1024],
                                    kt_t[64:128, 128 * kt : 128 * (kt + 1)],
                                    qt[64:128, qs + off : qs + 512],
                                    start=True,
                                    stop=True,
                                )
                                # exp on ACT, both heads in one instruction
                                st2 = strips[:, s, :].rearrange(
                                    "p (h w) -> p h w", h=2
                                )
                                nc.scalar.activation(
                                    P[:, :, off:512],
                                    st2[:, :, off:512],
                                    AF.Exp,
                                    scale=SCALE,
                                )
                                if diag:
                                    w = 128 * (m + 1)
                                    for hh in range(2):
                                        nc.gpsimd.tensor_mul(
                                            P[:, hh, 0:w],
                                            P[:, hh, 0:w],
                                            masks4[:, m, 0:w],
                                        )
                                # PV accumulate (full width; masked cols are 0)
                                for hh, h in ((0, hA), (1, hB)):
                                    nc.tensor.matmul(
                                        accm[0:65, hh, :],
                                        v_sb[kt][:, h, :],
                                        P[:, hh, :],
                                        start=(kt == 0),
                                        stop=(kt == nkt_q - 1),
                                    )
                                # interleave next-pair qk chunks across tasks
                                ti += 1
                                while qi < nqn and ti * nqn >= (qi + 1) * ntask:
                                    jt, tb = qn_chunks[qi]
                                    qi += 1
                                    if jt == 5 + p and tb + 2 < NTB:
                                        xr_dma((p, tb + 2), tb + 2)
                                    qps = qnps.tile(
                                        [128, 512], F32, tag="qn", name="qps"
                                    )
                                    for cb in range(NCB):
                                        nc.tensor.matmul(
                                            qps,
                                            wqk_sl[jt][:, cb, :],
                                            xring[(p, tb)][cb],
                                            start=(cb == 0),
                                            stop=(cb == NCB - 1),
                                        )
                                    nc.vector.tensor_copy(
                                        big[jt][:, 512 * tb : 512 * (tb + 1)], qps
                                    )
                            # evacuate acc -> yraw (unnormalized) + denominators.
                            # Engines are lane-mapped, so the head-B rows
                            # (acc partitions 0-64 -> yraw 64-127) and the
                            # denominator gather (partition 64 -> partition r)
                            # go through a staging tile + SBUF-to-SBUF DMA.
                            yt = yraw[p]
                            jb = slice(512 * jq, 512 * (jq + 1))
                            stg = stgp.tile([128, 2, 512], F32R, tag="stg", name="stg")
                            nc.vector.tensor_copy(yt[0:64, jb], accm[0:64, 0, :])
                            nc.vector.tensor_copy(stg[0:65, 0, :], accm[0:65, 1, :])
                            nc.vector.tensor_copy(stg[64:65, 1, :], accm[64:65, 0, :])
                            nc.sync.dma_start(yt[64:128, jb], stg[0:64, 0, :])
                            rA = 8 * p + jq
                            rB = 8 * p + 4 + jq
                            nc.sync.dma_start(
                                dens32[rB : rB + 1, :],
                                stg[64:65, 0, :].bitcast(F32),
                            )
                            nc.sync.dma_start(
                                dens32[rA : rA + 1, :],
                                stg[64:65, 1, :].bitcast(F32),
                            )


            # ---- epilogue: normalize + projection ----
            with (
                tc.tile_pool(name="wpp", bufs=1) as wpp,
                tc.tile_pool(name="otp", bufs=3) as otp,
                tc.tile_pool(name="bcps", bufs=2, space="PSUM") as bcpsp,
                tc.tile_pool(name="ps3", bufs=4, space="PSUM") as ps3,
            ):
                wp_sb = [
                    wpp.tile([128, C], F32R, tag=f"wp{j}", name=f"wp{j}")
                    for j in range(4)
                ]
                for jc in range(4):
                    nc.sync.dma_start(
                        wp_sb[jc], wp_d[128 * jc : 128 * (jc + 1), :].bitcast(F32R)
                    )
                # one reciprocal for all 32 denominator rows (DVE needs a
                # 32-aligned partition base, so per-pair slices are illegal)
                with nc.allow_low_precision(reason="softmax denom recip"):
                    nc.vector.reciprocal(recs32[0:32, :], dens32[0:32, :])
                for jq in range(4):
                    jb = slice(512 * jq, 512 * (jq + 1))
                    for p in range(4):
                        bc = bcpsp.tile([128, 512], F32, tag="bc", name="bc")
                        nc.tensor.matmul(
                            bc,
                            sel3[:, 4 * p + jq, :],
                            recs32[0:32, :],
                            start=True,
                            stop=True,
                        )
                        nc.vector.tensor_mul(yraw[p][:, jb], yraw[p][:, jb], bc)
                    for tt in range(4 * jq, 4 * jq + 4):
                        ot = otp.tile([128, C], F32, tag="ot", name="ot")
                        for nb in (0, 1):
                            ps = ps3.tile([128, 512], F32, tag="ps3", name="ps")
                            for jc in range(4):
                                nc.tensor.matmul(
                                    ps,
                                    yraw[jc][:, 128 * tt : 128 * (tt + 1)],
                                    wp_sb[jc][:, 512 * nb : 512 * (nb + 1)],
                                    start=(jc == 0),
                                    stop=(jc == 3),
                                )
                            nc.scalar.copy(ot[:, 512 * nb : 512 * (nb + 1)], ps)
                        nc.sync.dma_start(out_d[128 * tt : 128 * (tt + 1), :], ot)

    nc.compile()
    _cache["nc"] = nc
    return nc


def make_masks():
    # masks4[m][r, c] = 1 iff q-col c >= key row 128*m + r (within a 512 block)
    r = np.arange(128)[:, None]
    c = np.arange(512)[None, :]
    ms = [(c >= 128 * m + r).astype(np.float32) for m in range(4)]
    return np.concatenate(ms, axis=1)  # [128, 4*512]


def make_sel():
    # sel3[k, 4*p+jq, c] selects denominator row rA=8p+jq for out partitions
    # c<64 (head A) and rB=8p+4+jq for c>=64 (head B) in one bc matmul.
    s = np.zeros((32, 16, 128), np.float32)
    for p in range(4):
        for jq in range(4):
            s[8 * p + jq, 4 * p + jq, 0:64] = 1.0
            s[8 * p + 4 + jq, 4 * p + jq, 64:128] = 1.0
    return s.reshape(32, 16 * 128)


def make_in_maps(x, w_qkv, w_proj):
    masks = make_masks()
    sel = make_sel()
    ones = np.ones((128, HG), np.float32)
    wq, wk, wv = w_qkv[:, :C], w_qkv[:, C : 2 * C], w_qkv[:, 2 * C :]
    in_maps = []
    for c in range(8):
        b, g = divmod(c, 2)
        hs = slice(512 * g, 512 * (g + 1))
        in_maps.append(
            {
                "xt": np.ascontiguousarray(np.asarray(x[b]).T),
                "wqk": np.ascontiguousarray(
                    np.concatenate([wq[:, hs], wk[:, hs]], axis=1)
                ),
                "wv": np.ascontiguousarray(wv[:, hs]),
                "wp": np.ascontiguousarray(w_proj[512 * g : 512 * (g + 1), :]),
                "masks": masks,
                "sel": sel,
                "ones": ones,
            }
        )
    return in_maps


def kernel(x, w_qkv, w_proj):
    x = np.asarray(x, dtype=np.float32)
    w_qkv = np.asarray(w_qkv, dtype=np.float32)
    w_proj = np.asarray(w_proj, dtype=np.float32)
    nc = build_nc()
    in_maps = make_in_maps(x, w_qkv, w_proj)
    res = run_bass_kernel_spmd(nc, in_maps, core_ids=list(range(8)))
    out = np.empty((B, T, C), np.float32)
    for b in range(B):
        out[b] = res.results[2 * b]["out"] + res.results[2 * b + 1]["out"]
    return out


# revision 14
# speedup vs baseline: 1.2034x; 1.2034x over previous
"""Causal self-attention on 8 trn2 cores — v3.

Sharding: core c = 2*b + g handles batch b (of 4) and head group g (of 2,
8 heads each).  Tensor-parallel over heads for qkv/proj; host sums the two
w_proj partials per batch.

Structure: heads processed in PAIRS (head 2p at partitions 0-63, head 2p+1
at 64-127 of the qkT tiles), with

  - QK matmuls row-packed: the two K=64 stationaries sit at base partition
    0 / 64 so they land in disjoint PE array row groups and compute
    concurrently.
  - attention loop per (pair, q-block jq of 512, key tile kt of 128):
    QK -> exp (narrowed to causal-valid cols, both heads in one ACT
    instruction) -> 128-wide triangle mask on gpsimd -> PV accumulate
    (narrowed on diagonal tiles) into acc[65, 512]; the 65th row collects
    the softmax denominator via a ones column in v.
  - softmax normalization DEFERRED: denominator rows gathered by SBUF-SBUF
    DMA into a [32, 512] tile, one reciprocal, broadcast by a selector
    matmul, applied by one DVE multiply per (pair, jq) in the epilogue.
  - the NEXT pair's qkT matmuls run at jq boundaries (hiding the acc
    evacuation latency); x^T streams through a 2-deep ring of per-t-block
    mega tiles and is re-streamed per window.
  - yraw tiles reuse dead kT tiles (SBUF budget).

All matmul operands float32r.
"""

import numpy as np

import concourse.bacc as bacc
import concourse.bass as bass
import concourse.tile as tile
import concourse.mybir as mybir
from concourse.bass_utils import run_bass_kernel_spmd

F32 = mybir.dt.float32
F32R = mybir.dt.float32r
U32 = mybir.dt.uint32
AF = mybir.ActivationFunctionType

B, T, C, H = 4, 2048, 1024, 16
D = C // H  # 64
HG = 8  # heads per core
NCB = C // 128  # 8 contraction chunks
NTB = T // 512  # 4 t blocks
NKT = T // 128  # 16 key tiles
SCALE = 1.0 / 8.0  # 1/sqrt(D)

_cache = {}


def build_nc():
    if "nc" in _cache:
        return _cache["nc"]
    nc = bacc.Bacc("TRN2", target_bir_lowering=False, debug=False, num_devices=8)

    xt_d = nc.dram_tensor("xt", [C, T], F32, kind="ExternalInput").ap()
    wqk_d = nc.dram_tensor("wqk", [C, 2 * HG * D], F32, kind="ExternalInput").ap()
    wv_d = nc.dram_tensor("wv", [C, HG * D], F32, kind="ExternalInput").ap()
    wp_d = nc.dram_tensor("wp", [HG * D, C], F32, kind="ExternalInput").ap()
    masks_d = nc.dram_tensor("masks", [128, 128], F32, kind="ExternalInput").ap()
    sel_d = nc.dram_tensor("sel", [32, 16 * 128], F32, kind="ExternalInput").ap()
    ones_d = nc.dram_tensor("ones", [128, NKT * HG], F32, kind="ExternalInput").ap()
    out_d = nc.dram_tensor("out", [T, C], F32, kind="ExternalOutput").ap()

    with tile.TileContext(nc) as tc:
        with tc.tile_pool(name="persist", bufs=1) as persist:
            # big persistent tensors first; DMA triggers ordered so the first
            # matmuls' inputs arrive first.
            big = [
                persist.tile([128, T], F32R, tag=f"big{j}", name=f"big{j}")
                for j in range(9)
            ]
            qT = big[0:4]
            kT = big[4:8]
            yraw = [big[8], big[4], big[5], big[6]]

            v_sb = persist.tile([128, NKT, HG, 65], F32R, name="v_sb")
            tri = persist.tile([128, 128], F32R, name="tri")
            sel3 = persist.tile([32, 16, 128], F32R, name="sel3")
            dens32 = persist.tile([32, 512], F32, name="dens32")
            recs32 = persist.tile([32, 512], F32R, name="recs32")
            wp_sb = [
                persist.tile([128, C], F32R, tag=f"wp{j}", name=f"wp{j}")
                for j in range(4)
            ]

            with (
                tc.tile_pool(name="xrp", bufs=2) as xrp,
                tc.tile_pool(name="wqkp", bufs=4) as wqkp,
            ):
                xring = {}

                def xr_dma(key, tb):
                    t = xrp.tile([128, NCB, 512], F32R, tag="xr", name="xr")
                    nc.sync.dma_start(
                        t,
                        xt_d[:, 512 * tb : 512 * (tb + 1)]
                        .rearrange("(cb p) t -> p cb t", p=128)
                        .bitcast(F32R),
                    )
                    xring[key] = t

                wqk_sl = {}

                def dma_wqk(jt):
                    t = wqkp.tile([128, NCB, 128], F32R, tag="wqk", name=f"wqk{jt}")
                    nc.sync.dma_start(
                        t,
                        wqk_d[:, 128 * jt : 128 * (jt + 1)]
                        .rearrange("(cb p) j -> p cb j", p=128)
                        .bitcast(F32R),
                    )
                    wqk_sl[jt] = t

                with (
                    tc.tile_pool(name="wvp", bufs=1) as wvp,
                    tc.tile_pool(name="ps1", bufs=4, space="PSUM") as ps1,
                ):
                    # trigger order = arrival priority
                    xr_dma(("u", 0), 0)
                    dma_wqk(0)
                    dma_wqk(4)
                    wv_sb = wvp.tile([128, NCB, 512], F32R, name="wv_sb")
                    nc.sync.dma_start(
                        wv_sb, wv_d.rearrange("(cb p) j -> p cb j", p=128).bitcast(F32R)
                    )
                    xr_dma(("u", 1), 1)
                    nc.sync.dma_start(
                        v_sb[:, :, :, 64:65],
                        ones_d.rearrange("p (t h o) -> p t h o", t=NKT, o=1).bitcast(
                            F32R
                        ),
                    )
                    nc.sync.dma_start(tri, masks_d.bitcast(F32R))
                    nc.sync.dma_start(
                        sel3, sel_d.rearrange("p (r e) -> p r e", r=16).bitcast(F32R)
                    )
                    dma_wqk(1)
                    dma_wqk(5)
                    for jc in range(4):
                        nc.sync.dma_start(
                            wp_sb[jc], wp_d[128 * jc : 128 * (jc + 1), :].bitcast(F32R)
                        )

                    # ---- upfront: qk pair 0 + v for all heads, per tb ----
                    for tb in range(NTB):
                        if tb + 2 < NTB:
                            xr_dma(("u", tb + 2), tb + 2)
                        xc = xring[("u", tb)]
                        for jt in (0, 4):
                            ps = ps1.tile([128, 512], F32, tag="ps1", name="ps")
                            for cb in range(NCB):
                                nc.tensor.matmul(
                                    ps,
                                    wqk_sl[jt][:, cb, :],
                                    xc[:, cb, :],
                                    start=(cb == 0),
                                    stop=(cb == NCB - 1),
                                )
                            nc.vector.tensor_copy(
                                big[jt][:, 512 * tb : 512 * (tb + 1)], ps
                            )
                        for t4 in range(4):
                            tt = 4 * tb + t4
                            ps = ps1.tile([128, 512], F32, tag="ps1", name="ps")
                            for cb in range(NCB):
                                nc.tensor.matmul(
                                    ps,
                                    xc[:, cb, 128 * t4 : 128 * (t4 + 1)],
                                    wv_sb[:, cb, :],
                                    start=(cb == 0),
                                    stop=(cb == NCB - 1),
                                )
                            nc.scalar.copy(
                                v_sb[:, tt, :, 0:64],
                                ps[:].rearrange("p (h e) -> p h e", h=HG),
                            )

                # ---- attention: pairs of heads ----
                with (
                    tc.tile_pool(name="pp", bufs=2) as pp,
                    tc.tile_pool(name="stg", bufs=2) as stgp,
                    tc.tile_pool(name="strip", bufs=1, space="PSUM") as stripp,
                    tc.tile_pool(name="accp", bufs=1, space="PSUM") as accp,
                    tc.tile_pool(name="qnps", bufs=1, space="PSUM") as qnps,
                ):
                    strips = stripp.tile([128, 2, 1024], F32, name="strips")
                    accm = accp.tile([128, 2, 512], F32, name="accm")

                    for p in range(4):
                        hA, hB = 2 * p, 2 * p + 1
                        qt, kt_t = qT[p], kT[p]
                        if p < 2:
                            dma_wqk(p + 2)
                            dma_wqk(6 + p)
                        if p < 3:
                            xr_dma((p, 0), 0)
                            xr_dma((p, 1), 1)
                        si = 0
                        for jq in range(4):
                            nkt_q = 4 * jq + 4
                            for kt in range(nkt_q):
                                diag = (kt // 4) == jq
                                m = kt % 4 if diag else 0
                                off = 128 * m if diag else 0
                                s = si % 2
                                si += 1
                                P = pp.tile([128, 2, 512], F32R, tag="P", name="P")
                                # QK pair, row-packed (base partitions 0 / 64)
                                qs = 512 * jq
                                nc.tensor.matmul(
                                    strips[0:128, s, off:512],
                                    kt_t[0:64, 128 * kt : 128 * (kt + 1)],
                                    qt[0:64, qs + off : qs + 512],
                                    start=True,
                                    stop=True,
                                )
                                nc.tensor.matmul(
                                    strips[0:128, s, 512 + off : 1024],
                                    kt_t[64:128, 128 * kt : 128 * (kt + 1)],
                                    qt[64:128, qs + off : qs + 512],
                                    start=True,
                                    stop=True,
                                )
                                # exp on ACT, both heads in one instruction
                                st2 = strips[:, s, :].rearrange(
                                    "p (h w) -> p h w", h=2
                                )
                                nc.scalar.activation(
                                    P[:, :, off:512],
                                    st2[:, :, off:512],
                                    AF.Exp,
                                    scale=SCALE,
                                )
                                if diag:
                                    for hh in range(2):
                                        nc.gpsimd.tensor_mul(
                                            P[:, hh, off : off + 128],
                                            P[:, hh, off : off + 128],
                                            tri,
                                        )
                                # PV accumulate, narrowed on diagonal tiles
                                for hh, h in ((0, hA), (1, hB)):
                                    nc.tensor.matmul(
                                        accm[0:65, hh, off:512],
                                        v_sb[:, kt, h, :],
                                        P[:, hh, off:512],
                                        start=(kt == 0),
                                        stop=(kt == nkt_q - 1),
                                        skip_group_check=True,
                                    )
                            # jq boundary: evacuate acc while the PE runs the
                            # next pair's qk chunks (fills the evac latency)
                            yt = yraw[p]
                            jb = slice(512 * jq, 512 * (jq + 1))
                            stg = stgp.tile([128, 2, 512], F32R, tag="stg", name="stg")
                            nc.vector.tensor_copy(yt[0:64, jb], accm[0:64, 0, :])
                            nc.vector.tensor_copy(stg[0:65, 0, :], accm[0:65, 1, :])
                            nc.vector.tensor_copy(stg[64:65, 1, :], accm[64:65, 0, :])
                            if p < 3:
                                for jt in (p + 1, 5 + p):
                                    qps = qnps.tile(
                                        [128, 512], F32, tag="qn", name="qps"
                                    )
                                    for cb in range(NCB):
                                        nc.tensor.matmul(
                                            qps,
                                            wqk_sl[jt][:, cb, :],
                                            xring[(p, jq)][:, cb, :],
                                            start=(cb == 0),
                                            stop=(cb == NCB - 1),
                                        )
                                    nc.vector.tensor_copy(
                                        big[jt][:, jb], qps
                                    )
                                if jq + 2 < NTB:
                                    xr_dma((p, jq + 2), jq + 2)
                            nc.gpsimd.dma_start(yt[64:128, jb], stg[0:64, 0, :])
                            rA = 8 * p + jq
                            rB = 8 * p + 4 + jq
                            nc.gpsimd.dma_start(
                                dens32[rB : rB + 1, :], stg[64:65, 0, :].bitcast(F32)
                            )
                            nc.gpsimd.dma_start(
                                dens32[rA : rA + 1, :], stg[64:65, 1, :].bitcast(F32)
                            )
                    with nc.allow_low_precision(reason="softmax denom recip"):
                        nc.vector.reciprocal(recs32[0:32, :], dens32[0:32, :])

            # ---- epilogue: normalize + projection ----
            with (
                tc.tile_pool(name="otp", bufs=3) as otp,
                tc.tile_pool(name="bcps", bufs=2, space="PSUM") as bcpsp,
                tc.tile_pool(name="ps3", bufs=4, space="PSUM") as ps3,
            ):
                for jq in range(4):
                    jb = slice(512 * jq, 512 * (jq + 1))
                    for p in range(4):
                        bc = bcpsp.tile([128, 512], F32, tag="bc", name="bc")
                        nc.tensor.matmul(
                            bc,
                            sel3[:, 4 * p + jq, :],
                            recs32[0:32, :],
                            start=True,
                            stop=True,
                        )
                        nc.vector.tensor_mul(yraw[p][:, jb], yraw[p][:, jb], bc)
                    for tt in range(4 * jq, 4 * jq + 4):
                        ot = otp.tile([128, C], F32, tag="ot", name="ot")
                        for nb in (0, 1):
                            ps = ps3.tile([128, 512], F32, tag="ps3", name="ps")
                            for jc in range(4):
                                nc.tensor.matmul(
                                    ps,
                                    yraw[jc][:, 128 * tt : 128 * (tt + 1)],
                                    wp_sb[jc][:, 512 * nb : 512 * (nb + 1)],
                                    start=(jc == 0),
                                    stop=(jc == 3),
                                )
                            nc.scalar.copy(ot[:, 512 * nb : 512 * (nb + 1)], ps)
                        nc.sync.dma_start(out_d[128 * tt : 128 * (tt + 1), :], ot)

    nc.compile()
    _cache["nc"] = nc
    return nc


def make_masks():
    # tri[r, c] = 1 iff c >= r (within the 128-wide diagonal block)
    r = np.arange(128)[:, None]
    c = np.arange(128)[None, :]
    return (c >= r).astype(np.float32)


def make_sel():
    # sel3[k, 4*p+jq, c] selects denominator row rA=8p+jq for out partitions
    # c<64 (head A) and rB=8p+4+jq for c>=64 (head B) in one bc matmul.
    s = np.zeros((32, 16, 128), np.float32)
    for p in range(4):
        for jq in range(4):
            s[8 * p + jq, 4 * p + jq, 0:64] = 1.0
            s[8 * p + 4 + jq, 4 * p + jq, 64:128] = 1.0
    return s.reshape(32, 16 * 128)


def make_in_maps(x, w_qkv, w_proj):
    masks = make_masks()
    sel = make_sel()
    ones = np.ones((128, NKT * HG), np.float32)
    wq, wk, wv = w_qkv[:, :C], w_qkv[:, C : 2 * C], w_qkv[:, 2 * C :]
    in_maps = []
    for c in range(8):
        b, g = divmod(c, 2)
        hs = slice(512 * g, 512 * (g + 1))
        in_maps.append(
            {
                "xt": np.ascontiguousarray(np.asarray(x[b]).T),
                "wqk": np.ascontiguousarray(
                    np.concatenate([wq[:, hs], wk[:, hs]], axis=1)
                ),
                "wv": np.ascontiguousarray(wv[:, hs]),
                "wp": np.ascontiguousarray(w_proj[512 * g : 512 * (g + 1), :]),
                "masks": masks,
                "sel": sel,
                "ones": ones,
            }
        )
    return in_maps


def kernel(x, w_qkv, w_proj):
    x = np.asarray(x, dtype=np.float32)
    w_qkv = np.asarray(w_qkv, dtype=np.float32)
    w_proj = np.asarray(w_proj, dtype=np.float32)
    nc = build_nc()
    in_maps = make_in_maps(x, w_qkv, w_proj)
    res = run_bass_kernel_spmd(nc, in_maps, core_ids=list(range(8)))
    out = np.empty((B, T, C), np.float32)
    for b in range(B):
        out[b] = res.results[2 * b]["out"] + res.results[2 * b + 1]["out"]
    return out


# revision 17
# speedup vs baseline: 1.4241x; 1.1834x over previous
"""Causal self-attention on 8 trn2 cores — v3.

Sharding: core c = 2*b + g handles batch b (of 4) and head group g (of 2,
8 heads each).  Tensor-parallel over heads for qkv/proj; host sums the two
w_proj partials per batch.

Structure: heads processed in PAIRS (head 2p at partitions 0-63, head 2p+1
at 64-127 of the qkT tiles), with

  - QK matmuls row-packed: the two K=64 stationaries sit at base partition
    0 / 64 so they land in disjoint PE array row groups and compute
    concurrently.
  - attention loop per (pair, q-block jq of 512, key tile kt of 128):
    QK -> exp (narrowed to causal-valid cols, both heads in one ACT
    instruction) -> 128-wide triangle mask on gpsimd -> PV accumulate
    (narrowed on diagonal tiles) into acc[65, 512]; the 65th row collects
    the softmax denominator via a ones column in v.
  - softmax normalization DEFERRED: denominator rows gathered by SBUF-SBUF
    DMA into a [32, 512] tile, one reciprocal, broadcast by a selector
    matmul, applied by one DVE multiply per (pair, jq) in the epilogue.
  - the NEXT pair's qkT matmuls run at jq boundaries (hiding the acc
    evacuation latency); x^T streams through a 2-deep ring of per-t-block
    mega tiles and is re-streamed per window.
  - yraw tiles reuse dead kT tiles (SBUF budget).

All matmul operands float32r.
"""

import numpy as np

import concourse.bacc as bacc
import concourse.bass as bass
import concourse.tile as tile
import concourse.mybir as mybir
from concourse.bass_utils import run_bass_kernel_spmd

F32 = mybir.dt.float32
F32R = mybir.dt.float32r
U32 = mybir.dt.uint32
AF = mybir.ActivationFunctionType

B, T, C, H = 4, 2048, 1024, 16
D = C // H  # 64
HG = 8  # heads per core
NCB = C // 128  # 8 contraction chunks
NTB = T // 512  # 4 t blocks
NKT = T // 128  # 16 key tiles
SCALE = 1.0 / 8.0  # 1/sqrt(D)

_cache = {}


def build_nc():
    if "nc" in _cache:
        return _cache["nc"]
    nc = bacc.Bacc("TRN2", target_bir_lowering=False, debug=False, num_devices=8)

    xt_d = nc.dram_tensor("xt", [C, T], F32, kind="ExternalInput").ap()
    wqk_d = nc.dram_tensor("wqk", [C, 2 * HG * D], F32, kind="ExternalInput").ap()
    wv_d = nc.dram_tensor("wv", [C, HG * D], F32, kind="ExternalInput").ap()
    wp_d = nc.dram_tensor("wp", [HG * D, C], F32, kind="ExternalInput").ap()
    masks_d = nc.dram_tensor("masks", [128, 128], F32, kind="ExternalInput").ap()
    sel_d = nc.dram_tensor("sel", [32, 16 * 128], F32, kind="ExternalInput").ap()
    ones_d = nc.dram_tensor("ones", [128, NKT * HG], F32, kind="ExternalInput").ap()
    out_d = nc.dram_tensor("out", [T, C], F32, kind="ExternalOutput").ap()

    with tile.TileContext(nc) as tc:
        with tc.tile_pool(name="persist", bufs=1) as persist:
            # big persistent tensors first; DMA triggers ordered so the first
            # matmuls' inputs arrive first.
            big = [
                persist.tile([128, T], F32R, tag=f"big{j}", name=f"big{j}")
                for j in range(9)
            ]
            qT = big[0:4]
            kT = big[4:8]
            yraw = [big[8], big[4], big[5], big[6]]

            v_sb = persist.tile([128, NKT, HG, 65], F32R, name="v_sb")
            tri = persist.tile([128, 128], F32R, name="tri")
            sel3 = persist.tile([32, 16, 128], F32R, name="sel3")
            dens32 = persist.tile([32, 512], F32, name="dens32")
            recs32 = persist.tile([32, 512], F32R, name="recs32")
            wp_sb = [
                persist.tile([128, C], F32R, tag=f"wp{j}", name=f"wp{j}")
                for j in range(4)
            ]

            with (
                tc.tile_pool(name="xrp", bufs=2) as xrp,
                tc.tile_pool(name="wqkp", bufs=4) as wqkp,
            ):
                xring = {}

                def xr_dma(key, tb):
                    t = xrp.tile([128, NCB, 512], F32R, tag="xr", name="xr")
                    nc.sync.dma_start(
                        t,
                        xt_d[:, 512 * tb : 512 * (tb + 1)]
                        .rearrange("(cb p) t -> p cb t", p=128)
                        .bitcast(F32R),
                    )
                    xring[key] = t

                wqk_sl = {}

                def dma_wqk(jt):
                    t = wqkp.tile([128, NCB, 128], F32R, tag="wqk", name=f"wqk{jt}")
                    nc.sync.dma_start(
                        t,
                        wqk_d[:, 128 * jt : 128 * (jt + 1)]
                        .rearrange("(cb p) j -> p cb j", p=128)
                        .bitcast(F32R),
                    )
                    wqk_sl[jt] = t

                with (
                    tc.tile_pool(name="wvp", bufs=1) as wvp,
                    tc.tile_pool(name="ps1", bufs=4, space="PSUM") as ps1,
                ):
                    # trigger order = arrival priority
                    xr_dma(("u", 0), 0)
                    dma_wqk(0)
                    dma_wqk(4)
                    wv_sb = wvp.tile([128, NCB, 512], F32R, name="wv_sb")
                    nc.sync.dma_start(
                        wv_sb, wv_d.rearrange("(cb p) j -> p cb j", p=128).bitcast(F32R)
                    )
                    xr_dma(("u", 1), 1)
                    nc.sync.dma_start(
                        v_sb[:, :, :, 64:65],
                        ones_d.rearrange("p (t h o) -> p t h o", t=NKT, o=1).bitcast(
                            F32R
                        ),
                    )
                    nc.sync.dma_start(tri, masks_d.bitcast(F32R))
                    nc.sync.dma_start(
                        sel3, sel_d.rearrange("p (r e) -> p r e", r=16).bitcast(F32R)
                    )
                    dma_wqk(1)
                    dma_wqk(5)
                    for jc in range(4):
                        nc.sync.dma_start(
                            wp_sb[jc], wp_d[128 * jc : 128 * (jc + 1), :].bitcast(F32R)
                        )

                    # ---- upfront: v for all heads + qk pair 0, per tb ----
                    # (v first so the last PSUM readers are fast DVE copies,
                    # clearing the bank-reuse WAR for the attention strips)
                    for tb in range(NTB):
                        if tb + 2 < NTB:
                            xr_dma(("u", tb + 2), tb + 2)
                        xc = xring[("u", tb)]
                        for t4 in range(4):
                            tt = 4 * tb + t4
                            ps = ps1.tile([128, 512], F32, tag="ps1", name="ps")
                            for cb in range(NCB):
                                nc.tensor.matmul(
                                    ps,
                                    xc[:, cb, 128 * t4 : 128 * (t4 + 1)],
                                    wv_sb[:, cb, :],
                                    start=(cb == 0),
                                    stop=(cb == NCB - 1),
                                )
                            nc.scalar.copy(
                                v_sb[:, tt, :, 0:64],
                                ps[:].rearrange("p (h e) -> p h e", h=HG),
                            )
                        for jt in (0, 4):
                            ps = ps1.tile([128, 512], F32, tag="ps1", name="ps")
                            for cb in range(NCB):
                                nc.tensor.matmul(
                                    ps,
                                    wqk_sl[jt][:, cb, :],
                                    xc[:, cb, :],
                                    start=(cb == 0),
                                    stop=(cb == NCB - 1),
                                )
                            nc.vector.tensor_copy(
                                big[jt][:, 512 * tb : 512 * (tb + 1)], ps
                            )

                # ---- attention: pairs of heads ----
                with (
                    tc.tile_pool(name="pp", bufs=4) as pp,
                    tc.tile_pool(name="stg", bufs=2) as stgp,
                    tc.tile_pool(name="strip", bufs=1, space="PSUM") as stripp,
                    tc.tile_pool(name="accp", bufs=1, space="PSUM") as accp,
                    tc.tile_pool(name="qnps", bufs=1, space="PSUM") as qnps,
                ):
                    strips = stripp.tile([128, 2, 1024], F32, name="strips")
                    accm = accp.tile([128, 2, 512], F32, name="accm")
                    LAG = 3  # PV trails QK by LAG tiles so exp latency hides

                    for p in range(4):
                        hA, hB = 2 * p, 2 * p + 1
                        qt, kt_t = qT[p], kT[p]
                        if p < 2:
                            dma_wqk(p + 2)
                            dma_wqk(6 + p)
                        if p < 3:
                            xr_dma((p, 0), 0)
                            xr_dma((p, 1), 1)
                        si = 0
                        for jq in range(4):
                            nkt_q = 4 * jq + 4
                            qs = 512 * jq
                            pend = []  # (kt, P, off) awaiting their PV

                            def emit_pv(kt, P, off):
                                for hh, h in ((0, hA), (1, hB)):
                                    nc.tensor.matmul(
                                        accm[0:65, hh, off:512],
                                        v_sb[:, kt, h, :],
                                        P[:, hh, off:512],
                                        start=(kt == 0),
                                        stop=(kt == nkt_q - 1),
                                        skip_group_check=True,
                                    )

                            for kt in range(nkt_q):
                                diag = (kt // 4) == jq
                                m = kt % 4 if diag else 0
                                off = 128 * m if diag else 0
                                s = si % 2
                                si += 1
                                P = pp.tile([128, 2, 512], F32R, tag="P", name="P")
                                # QK pair, row-packed (base partitions 0 / 64)
                                nc.tensor.matmul(
                                    strips[0:128, s, off:512],
                                    kt_t[0:64, 128 * kt : 128 * (kt + 1)],
                                    qt[0:64, qs + off : qs + 512],
                                    start=True,
                                    stop=True,
                                )
                                nc.tensor.matmul(
                                    strips[0:128, s, 512 + off : 1024],
                                    kt_t[64:128, 128 * kt : 128 * (kt + 1)],
                                    qt[64:128, qs + off : qs + 512],
                                    start=True,
                                    stop=True,
                                )
                                # exp on ACT, both heads in one instruction
                                st2 = strips[:, s, :].rearrange(
                                    "p (h w) -> p h w", h=2
                                )
                                nc.scalar.activation(
                                    P[:, :, off:512],
                                    st2[:, :, off:512],
                                    AF.Exp,
                                    scale=SCALE,
                                )
                                if diag:
                                    for hh in range(2):
                                        nc.gpsimd.tensor_mul(
                                            P[:, hh, off : off + 128],
                                            P[:, hh, off : off + 128],
                                            tri,
                                        )
                                pend.append((kt, P, off))
                                if len(pend) > LAG:
                                    emit_pv(*pend.pop(0))
                            for args in pend:
                                emit_pv(*args)
                            # jq boundary: evacuate acc while the PE runs the
                            # next pair's qk chunks (fills the evac latency)
                            yt = yraw[p]
                            jb = slice(512 * jq, 512 * (jq + 1))
                            stg = stgp.tile([128, 2, 512], F32R, tag="stg", name="stg")
                            nc.vector.tensor_copy(yt[0:64, jb], accm[0:64, 0, :])
                            nc.vector.tensor_copy(stg[0:65, 0, :], accm[0:65, 1, :])
                            nc.vector.tensor_copy(stg[64:65, 1, :], accm[64:65, 0, :])
                            if p < 3:
                                for jt in (p + 1, 5 + p):
                                    qps = qnps.tile(
                                        [128, 512], F32, tag="qn", name="qps"
                                    )
                                    for cb in range(NCB):
                                        nc.tensor.matmul(
                                            qps,
                                            wqk_sl[jt][:, cb, :],
                                            xring[(p, jq)][:, cb, :],
                                            start=(cb == 0),
                                            stop=(cb == NCB - 1),
                                        )
                                    nc.vector.tensor_copy(
                                        big[jt][:, jb], qps
                                    )
                                if jq + 2 < NTB:
                                    xr_dma((p, jq + 2), jq + 2)
                            nc.gpsimd.dma_start(yt[64:128, jb], stg[0:64, 0, :])
                            rA = 8 * p + jq
                            rB = 8 * p + 4 + jq
                            nc.gpsimd.dma_start(
                                dens32[rB : rB + 1, :], stg[64:65, 0, :].bitcast(F32)
                            )
                            nc.gpsimd.dma_start(
                                dens32[rA : rA + 1, :], stg[64:65, 1, :].bitcast(F32)
                            )
                    with nc.allow_low_precision(reason="softmax denom recip"):
                        nc.vector.reciprocal(recs32[0:32, :], dens32[0:32, :])

            # ---- epilogue: normalize + projection ----
            with (
                tc.tile_pool(name="otp", bufs=3) as otp,
                tc.tile_pool(name="bcps", bufs=2, space="PSUM") as bcpsp,
                tc.tile_pool(name="ps3", bufs=4, space="PSUM") as ps3,
            ):
                for jq in range(4):
                    jb = slice(512 * jq, 512 * (jq + 1))
                    for p in range(4):
                        bc = bcpsp.tile([128, 512], F32, tag="bc", name="bc")
                        nc.tensor.matmul(
                            bc,
                            sel3[:, 4 * p + jq, :],
                            recs32[0:32, :],
                            start=True,
                            stop=True,
                        )
                        nc.vector.tensor_mul(yraw[p][:, jb], yraw[p][:, jb], bc)
                    for tt in range(4 * jq, 4 * jq + 4):
                        ot = otp.tile([128, C], F32, tag="ot", name="ot")
                        for nb in (0, 1):
                            ps = ps3.tile([128, 512], F32, tag="ps3", name="ps")
                            for jc in range(4):
                                nc.tensor.matmul(
                                    ps,
                                    yraw[jc][:, 128 * tt : 128 * (tt + 1)],
                                    wp_sb[jc][:, 512 * nb : 512 * (nb + 1)],
                                    start=(jc == 0),
                                    stop=(jc == 3),
                                )
                            nc.scalar.copy(ot[:, 512 * nb : 512 * (nb + 1)], ps)
                        nc.sync.dma_start(out_d[128 * tt : 128 * (tt + 1), :], ot)

    nc.compile()
    _cache["nc"] = nc
    return nc


def make_masks():
    # tri[r, c] = 1 iff c >= r (within the 128-wide diagonal block)
    r = np.arange(128)[:, None]
    c = np.arange(128)[None, :]
    return (c >= r).astype(np.float32)


def make_sel():
    # sel3[k, 4*p+jq, c] selects denominator row rA=8p+jq for out partitions
    # c<64 (head A) and rB=8p+4+jq for c>=64 (head B) in one bc matmul.
    s = np.zeros((32, 16, 128), np.float32)
    for p in range(4):
        for jq in range(4):
            s[8 * p + jq, 4 * p + jq, 0:64] = 1.0
            s[8 * p + 4 + jq, 4 * p + jq, 64:128] = 1.0
    return s.reshape(32, 16 * 128)


def make_in_maps(x, w_qkv, w_proj):
    masks = make_masks()
    sel = make_sel()
    ones = np.ones((128, NKT * HG), np.float32)
    wq, wk, wv = w_qkv[:, :C], w_qkv[:, C : 2 * C], w_qkv[:, 2 * C :]
    in_maps = []
    for c in range(8):
        b, g = divmod(c, 2)
        hs = slice(512 * g, 512 * (g + 1))
        in_maps.append(
            {
                "xt": np.ascontiguousarray(np.asarray(x[b]).T),
                "wqk": np.ascontiguousarray(
                    np.concatenate([wq[:, hs], wk[:, hs]], axis=1)
                ),
                "wv": np.ascontiguousarray(wv[:, hs]),
                "wp": np.ascontiguousarray(w_proj[512 * g : 512 * (g + 1), :]),
                "masks": masks,
                "sel": sel,
                "ones": ones,
            }
        )
    return in_maps


def kernel(x, w_qkv, w_proj):
    x = np.asarray(x, dtype=np.float32)
    w_qkv = np.asarray(w_qkv, dtype=np.float32)
    w_proj = np.asarray(w_proj, dtype=np.float32)
    nc = build_nc()
    in_maps = make_in_maps(x, w_qkv, w_proj)
    res = run_bass_kernel_spmd(nc, in_maps, core_ids=list(range(8)))
    out = np.empty((B, T, C), np.float32)
    for b in range(B):
        out[b] = res.results[2 * b]["out"] + res.results[2 * b + 1]["out"]
    return out


# revision 18
# speedup vs baseline: 1.6726x; 1.1745x over previous
"""Causal self-attention on 8 trn2 cores — v4 (bf16 operands).

Sharding: core c = 2*b + g handles batch b (of 4) and head group g (of 2,
8 heads each).  Tensor-parallel over heads for qkv/proj; host sums the two
w_proj partials per batch.

Structure: heads processed in PAIRS (head 2p at partitions 0-63, head 2p+1
at 64-127 of the qkT tiles), with

  - QK matmuls row-packed: the two K=64 stationaries sit at base partition
    0 / 64 so they land in disjoint PE array row groups and compute
    concurrently.
  - attention loop per (pair, q-block jq of 512, key tile kt of 128):
    QK -> exp (narrowed to causal-valid cols, both heads in one ACT
    instruction) -> 128-wide triangle mask on gpsimd -> PV accumulate
    (narrowed on diagonal tiles, trailing QK by LAG tiles so the exp
    latency stays off the PE) into acc[65, 512]; the 65th row collects the
    softmax denominator via a ones column in v.
  - softmax normalization DEFERRED: denominator rows gathered by SBUF-SBUF
    DMA into a [32, 512] tile, one reciprocal, broadcast by a selector
    matmul, applied by one DVE multiply per (pair, jq) in the epilogue.
  - the NEXT pair's qkT matmuls run mid-jq (kt==2) as PE ballast so the
    tensor engine never idles (HAM keeps the clock at 2.4 GHz); x^T
    streams through a 2-deep ring of per-t-block mega tiles, re-streamed
    per window.
  - yraw tiles reuse dead kT tiles (SBUF budget).

All matmul operands bf16 (PSUM accumulation f32); exp reads the f32
logits, so only operand rounding costs precision (~1e-3 relative).
"""

import numpy as np
import ml_dtypes

import concourse.bacc as bacc
import concourse.bass as bass
import concourse.tile as tile
import concourse.mybir as mybir
from concourse.bass_utils import run_bass_kernel_spmd

F32 = mybir.dt.float32
BF16 = mybir.dt.bfloat16
AF = mybir.ActivationFunctionType

B, T, C, H = 4, 2048, 1024, 16
D = C // H  # 64
HG = 8  # heads per core
NCB = C // 128  # 8 contraction chunks
NTB = T // 512  # 4 t blocks
NKT = T // 128  # 16 key tiles
SCALE = 1.0 / 8.0  # 1/sqrt(D)

_cache = {}


def build_nc():
    if "nc" in _cache:
        return _cache["nc"]
    nc = bacc.Bacc("TRN2", target_bir_lowering=False, debug=False, num_devices=8)

    xt_d = nc.dram_tensor("xt", [C, T], BF16, kind="ExternalInput").ap()
    wqk_d = nc.dram_tensor("wqk", [C, 2 * HG * D], BF16, kind="ExternalInput").ap()
    wv_d = nc.dram_tensor("wv", [C, HG * D], BF16, kind="ExternalInput").ap()
    wp_d = nc.dram_tensor("wp", [HG * D, C], BF16, kind="ExternalInput").ap()
    masks_d = nc.dram_tensor("masks", [128, 128], BF16, kind="ExternalInput").ap()
    sel_d = nc.dram_tensor("sel", [32, 16 * 128], BF16, kind="ExternalInput").ap()
    ones_d = nc.dram_tensor("ones", [128, NKT * HG], BF16, kind="ExternalInput").ap()
    out_d = nc.dram_tensor("out", [T, C], F32, kind="ExternalOutput").ap()

    with tile.TileContext(nc) as tc:
        with tc.tile_pool(name="persist", bufs=1) as persist:
            big = [
                persist.tile([128, T], BF16, tag=f"big{j}", name=f"big{j}")
                for j in range(9)
            ]
            qT = big[0:4]
            kT = big[4:8]
            yraw = [big[8], big[4], big[5], big[6]]

            v_sb = persist.tile([128, NKT, HG, 65], BF16, name="v_sb")
            tri = persist.tile([128, 128], BF16, name="tri")
            sel3 = persist.tile([32, 16, 128], BF16, name="sel3")
            dens32 = persist.tile([32, 512], BF16, name="dens32")
            recs32 = persist.tile([32, 512], BF16, name="recs32")
            wp_sb = [
                persist.tile([128, C], BF16, tag=f"wp{j}", name=f"wp{j}")
                for j in range(4)
            ]

            with (
                tc.tile_pool(name="xrp", bufs=2) as xrp,
                tc.tile_pool(name="wqkp", bufs=4) as wqkp,
            ):
                xring = {}

                def xr_dma(key, tb):
                    t = xrp.tile([128, NCB, 512], BF16, tag="xr", name="xr")
                    nc.sync.dma_start(
                        t,
                        xt_d[:, 512 * tb : 512 * (tb + 1)].rearrange(
                            "(cb p) t -> p cb t", p=128
                        ),
                    )
                    xring[key] = t

                wqk_sl = {}

                def dma_wqk(jt):
                    t = wqkp.tile([128, NCB, 128], BF16, tag="wqk", name=f"wqk{jt}")
                    nc.sync.dma_start(
                        t,
                        wqk_d[:, 128 * jt : 128 * (jt + 1)].rearrange(
                            "(cb p) j -> p cb j", p=128
                        ),
                    )
                    wqk_sl[jt] = t

                with (
                    tc.tile_pool(name="wvp", bufs=1) as wvp,
                    tc.tile_pool(name="ps1", bufs=4, space="PSUM") as ps1,
                ):
                    # trigger order = arrival priority
                    xr_dma(("u", 0), 0)
                    dma_wqk(0)
                    dma_wqk(4)
                    wv_sb = wvp.tile([128, NCB, 512], BF16, name="wv_sb")
                    nc.sync.dma_start(
                        wv_sb, wv_d.rearrange("(cb p) j -> p cb j", p=128)
                    )
                    xr_dma(("u", 1), 1)
                    nc.sync.dma_start(
                        v_sb[:, :, :, 64:65],
                        ones_d.rearrange("p (t h o) -> p t h o", t=NKT, o=1),
                    )
                    nc.sync.dma_start(tri, masks_d)
                    nc.sync.dma_start(
                        sel3, sel_d.rearrange("p (r e) -> p r e", r=16)
                    )
                    dma_wqk(1)
                    dma_wqk(5)
                    for jc in range(4):
                        nc.sync.dma_start(
                            wp_sb[jc], wp_d[128 * jc : 128 * (jc + 1), :]
                        )

                    # ---- upfront: v for all heads + qk pair 0, per tb ----
                    for tb in range(NTB):
                        if tb + 2 < NTB:
                            xr_dma(("u", tb + 2), tb + 2)
                        xc = xring[("u", tb)]
                        for t4 in range(4):
                            tt = 4 * tb + t4
                            ps = ps1.tile([128, 512], F32, tag="ps1", name="ps")
                            for cb in range(NCB):
                                nc.tensor.matmul(
                                    ps,
                                    xc[:, cb, 128 * t4 : 128 * (t4 + 1)],
                                    wv_sb[:, cb, :],
                                    start=(cb == 0),
                                    stop=(cb == NCB - 1),
                                )
                            nc.scalar.copy(
                                v_sb[:, tt, :, 0:64],
                                ps[:].rearrange("p (h e) -> p h e", h=HG),
                            )
                        for jt in (0, 4):
                            ps = ps1.tile([128, 512], F32, tag="ps1", name="ps")
                            for cb in range(NCB):
                                nc.tensor.matmul(
                                    ps,
                                    wqk_sl[jt][:, cb, :],
                                    xc[:, cb, :],
                                    start=(cb == 0),
                                    stop=(cb == NCB - 1),
                                )
                            nc.vector.tensor_copy(
                                big[jt][:, 512 * tb : 512 * (tb + 1)], ps
                            )

                # ---- attention: pairs of heads ----
                with (
                    tc.tile_pool(name="pp", bufs=4) as pp,
                    tc.tile_pool(name="stg", bufs=2) as stgp,
                    tc.tile_pool(name="strip", bufs=1, space="PSUM") as stripp,
                    tc.tile_pool(name="accp", bufs=1, space="PSUM") as accp,
                    tc.tile_pool(name="qnps", bufs=1, space="PSUM") as qnps,
                ):
                    strips = stripp.tile([128, 2, 1024], F32, name="strips")
                    accm = accp.tile([128, 2, 512], F32, name="accm")
                    LAG = 3  # PV trails QK by LAG tiles so exp latency hides

                    for p in range(4):
                        hA, hB = 2 * p, 2 * p + 1
                        qt, kt_t = qT[p], kT[p]
                        if p < 2:
                            dma_wqk(p + 2)
                            dma_wqk(6 + p)
                        if p < 3:
                            xr_dma((p, 0), 0)
                            xr_dma((p, 1), 1)
                        si = 0
                        for jq in range(4):
                            nkt_q = 4 * jq + 4
                            qs = 512 * jq
                            pend = []  # (kt, P, off) awaiting their PV

                            def emit_pv(kt, P, off):
                                for hh, h in ((0, hA), (1, hB)):
                                    nc.tensor.matmul(
                                        accm[0:65, hh, off:512],
                                        v_sb[:, kt, h, :],
                                        P[:, hh, off:512],
                                        start=(kt == 0),
                                        stop=(kt == nkt_q - 1),
                                        skip_group_check=True,
                                    )

                            def emit_qn(jt, tb):
                                qps = qnps.tile([128, 512], F32, tag="qn", name="qps")
                                for cb in range(NCB):
                                    nc.tensor.matmul(
                                        qps,
                                        wqk_sl[jt][:, cb, :],
                                        xring[(p, tb)][:, cb, :],
                                        start=(cb == 0),
                                        stop=(cb == NCB - 1),
                                    )
                                nc.vector.tensor_copy(
                                    big[jt][:, 512 * tb : 512 * (tb + 1)], qps
                                )

                            for kt in range(nkt_q):
                                diag = (kt // 4) == jq
                                m = kt % 4 if diag else 0
                                off = 128 * m if diag else 0
                                s = si % 2
                                si += 1
                                P = pp.tile([128, 2, 512], BF16, tag="P", name="P")
                                # QK pair, row-packed (base partitions 0 / 64)
                                nc.tensor.matmul(
                                    strips[0:128, s, off:512],
                                    kt_t[0:64, 128 * kt : 128 * (kt + 1)],
                                    qt[0:64, qs + off : qs + 512],
                                    start=True,
                                    stop=True,
                                )
                                nc.tensor.matmul(
                                    strips[0:128, s, 512 + off : 1024],
                                    kt_t[64:128, 128 * kt : 128 * (kt + 1)],
                                    qt[64:128, qs + off : qs + 512],
                                    start=True,
                                    stop=True,
                                )
                                # exp on ACT, both heads in one instruction
                                st2 = strips[:, s, :].rearrange(
                                    "p (h w) -> p h w", h=2
                                )
                                nc.scalar.activation(
                                    P[:, :, off:512],
                                    st2[:, :, off:512],
                                    AF.Exp,
                                    scale=SCALE,
                                )
                                if diag:
                                    for hh in range(2):
                                        nc.gpsimd.tensor_mul(
                                            P[:, hh, off : off + 128],
                                            P[:, hh, off : off + 128],
                                            tri,
                                        )
                                pend.append((kt, P, off))
                                if len(pend) > LAG:
                                    emit_pv(*pend.pop(0))
                                # next-pair qk ballast keeps the PE saturated
                                if kt == 2 and p < 3:
                                    emit_qn(p + 1, jq)
                                    emit_qn(5 + p, jq)
                            for args in pend:
                                emit_pv(*args)
                            # jq boundary: evacuate acc
                            yt = yraw[p]
                            jb = slice(512 * jq, 512 * (jq + 1))
                            stg = stgp.tile([128, 2, 512], BF16, tag="stg", name="stg")
                            nc.vector.tensor_copy(yt[0:64, jb], accm[0:64, 0, :])
                            nc.vector.tensor_copy(stg[0:65, 0, :], accm[0:65, 1, :])
                            nc.vector.tensor_copy(stg[64:65, 1, :], accm[64:65, 0, :])
                            if p < 3 and jq + 2 < NTB:
                                xr_dma((p, jq + 2), jq + 2)
                            nc.gpsimd.dma_start(yt[64:128, jb], stg[0:64, 0, :])
                            rA = 8 * p + jq
                            rB = 8 * p + 4 + jq
                            nc.gpsimd.dma_start(
                                dens32[rB : rB + 1, :], stg[64:65, 0, :]
                            )
                            nc.gpsimd.dma_start(
                                dens32[rA : rA + 1, :], stg[64:65, 1, :]
                            )
                    with nc.allow_low_precision(reason="softmax denom recip"):
                        nc.vector.reciprocal(recs32[0:32, :], dens32[0:32, :])

            # ---- epilogue: normalize + projection ----
            with (
                tc.tile_pool(name="otp", bufs=3) as otp,
                tc.tile_pool(name="bcps", bufs=2, space="PSUM") as bcpsp,
                tc.tile_pool(name="ps3", bufs=4, space="PSUM") as ps3,
            ):
                for jq in range(4):
                    jb = slice(512 * jq, 512 * (jq + 1))
                    for p in range(4):
                        bc = bcpsp.tile([128, 512], F32, tag="bc", name="bc")
                        nc.tensor.matmul(
                            bc,
                            sel3[:, 4 * p + jq, :],
                            recs32[0:32, :],
                            start=True,
                            stop=True,
                        )
                        nc.vector.tensor_mul(yraw[p][:, jb], yraw[p][:, jb], bc)
                    for tt in range(4 * jq, 4 * jq + 4):
                        ot = otp.tile([128, C], F32, tag="ot", name="ot")
                        for nb in (0, 1):
                            ps = ps3.tile([128, 512], F32, tag="ps3", name="ps")
                            for jc in range(4):
                                nc.tensor.matmul(
                                    ps,
                                    yraw[jc][:, 128 * tt : 128 * (tt + 1)],
                                    wp_sb[jc][:, 512 * nb : 512 * (nb + 1)],
                                    start=(jc == 0),
                                    stop=(jc == 3),
                                )
                            nc.scalar.copy(ot[:, 512 * nb : 512 * (nb + 1)], ps)
                        nc.sync.dma_start(out_d[128 * tt : 128 * (tt + 1), :], ot)

    nc.compile()
    _cache["nc"] = nc
    return nc


def make_masks():
    # tri[r, c] = 1 iff c >= r (within the 128-wide diagonal block)
    r = np.arange(128)[:, None]
    c = np.arange(128)[None, :]
    return (c >= r).astype(np.float32)


def make_sel():
    # sel3[k, 4*p+jq, c] selects denominator row rA=8p+jq for out partitions
    # c<64 (head A) and rB=8p+4+jq for c>=64 (head B) in one bc matmul.
    s = np.zeros((32, 16, 128), np.float32)
    for p in range(4):
        for jq in range(4):
            s[8 * p + jq, 4 * p + jq, 0:64] = 1.0
            s[8 * p + 4 + jq, 4 * p + jq, 64:128] = 1.0
    return s.reshape(32, 16 * 128)


def _bf(a):
    return np.ascontiguousarray(np.asarray(a, np.float32).astype(ml_dtypes.bfloat16))


def make_in_maps(x, w_qkv, w_proj):
    masks = _bf(make_masks())
    sel = _bf(make_sel())
    ones = _bf(np.ones((128, NKT * HG), np.float32))
    wq, wk, wv = w_qkv[:, :C], w_qkv[:, C : 2 * C], w_qkv[:, 2 * C :]
    in_maps = []
    for c in range(8):
        b, g = divmod(c, 2)
        hs = slice(512 * g, 512 * (g + 1))
        in_maps.append(
            {
                "xt": _bf(np.asarray(x[b]).T),
                "wqk": _bf(np.concatenate([wq[:, hs], wk[:, hs]], axis=1)),
                "wv": _bf(wv[:, hs]),
                "wp": _bf(w_proj[512 * g : 512 * (g + 1), :]),
                "masks": masks,
                "sel": sel,
                "ones": ones,
            }
        )
    return in_maps


def kernel(x, w_qkv, w_proj):
    x = np.asarray(x, dtype=np.float32)
    w_qkv = np.asarray(w_qkv, dtype=np.float32)
    w_proj = np.asarray(w_proj, dtype=np.float32)
    nc = build_nc()
    in_maps = make_in_maps(x, w_qkv, w_proj)
    res = run_bass_kernel_spmd(nc, in_maps, core_ids=list(range(8)))
    out = np.empty((B, T, C), np.float32)
    for b in range(B):
        out[b] = res.results[2 * b]["out"] + res.results[2 * b + 1]["out"]
    return out
